# revision 24
# baseline (speedup 1.0000x reference)
"""Trainium2 Bass kernel for nn_Aligner segment_reduce.

Computation: out = (segment_sum(embed_weight[flat_idx]) / lens) @ proj_w + proj_b
Shapes: flat_idx [65536], seg [65536] (sorted), lens [2048],
        embed_weight [50000, 3584], proj_w [3584, 128], proj_b [128].

Strategy (8 NeuronCores, data-parallel over segment-sorted tokens):
- Tokens are stably sorted by segment id; core k owns segments
  [256k, 256k+256) == two aligned 128-segment windows and gathers the
  full 3584-wide embedding rows of its tokens (fp16 copy of the table,
  ~62 MB of HBM reads per core).
- Vocab is split at 32768 so row ids fit int16 for the custom GPSIMD
  dma_gather instruction (two tables / two gather passes per window).
- Token sublists per (core, window, pass) are padded to global caps so
  every core runs the IDENTICAL program (SPMD) on different data.
- Each gather call fetches 4 columns x 128 rows.  Per 128-token column,
  an indicator matrix E[tok, seg_in_window] is built on-device
  (is_equal against an iota matrix, fp16) and E.T @ G accumulates
  segment sums for the window into a 7-bank [128, 3584] f32 PSUM tile.
- Per window: scale by 1/lens, transpose 128-wide chunks via matmul with
  identity, then GEMM with proj_w accumulating over the 28 chunks.
- Host assembles the per-core [256, 128] outputs and adds proj_b.
"""

import sys

sys.path.insert(0, "/opt/trn_rl_repo")

import numpy as np

T = 65536
B = 2048
V = 50000
D = 3584
DE = 128
NCORES = 8
SPLIT = 32768              # vocab split so indices fit int16
VLO, VHI = SPLIT, V - SPLIT
P = 128
CALL_COLS = 2              # 2 columns (256 indices) per dma_gather call
NW = 2                     # 128-seg windows per core
SEGS_PER_CORE = B // NCORES          # 256
NCHUNK = D // 512          # 7 psum-bank chunks for the E-matmuls
NTCH = D // P              # 28 transpose/proj chunks
PAD_SEG = -1000            # sentinel seg value for padded token slots

LAST_RESULTS = None        # BassKernelResults of the most recent run


def _ensure_axon_ntff_hook():
    """bass_utils imports antenv.axon_hooks when trace=True under axon;
    some images lack that module.  Provide it, wired to the libaxon ctypes
    NTFF profiler when available (else the hook stays None and bass_utils
    skips tracing gracefully)."""
    try:
        from antenv import axon_hooks  # noqa: F401
        return
    except ImportError:
        pass
    import types

    try:
        import antenv
    except ImportError:
        return
    mod = types.ModuleType("antenv.axon_hooks")
    _hook = [None]
    mod.set_axon_ntff_profile_hook = lambda h: _hook.__setitem__(0, h)
    mod.get_axon_ntff_profile_hook = lambda: _hook[0]
    sys.modules["antenv.axon_hooks"] = mod
    antenv.axon_hooks = mod
    try:
        if "/root/.axon_site" not in sys.path:
            sys.path.insert(0, "/root/.axon_site")
        from trn_agent_boot.trn_boot import _ntff_profile_via_ctypes

        mod.set_axon_ntff_profile_hook(
            _ntff_profile_via_ctypes("/opt/axon/libaxon_pjrt.so")
        )
    except Exception:
        pass


def _move_gather_waits(nc, mybir):
    """InstDMAGatherAnt cannot carry sem waits on HW (custom NX decode path
    wedges the device).  Move each gather's on_wait onto a fresh Pool
    InstNoOp inserted immediately before it."""
    n_moved = 0
    for f in nc.m.functions:
        for blk in f.blocks:
            new_insts = []
            for inst in blk.instructions:
                if (
                    isinstance(inst, mybir.InstDMAGatherAnt)
                    and inst.sync_info
                    and inst.sync_info.on_wait
                ):
                    nop = mybir.InstNoOp(
                        name=f"I-gwaitc-{n_moved}",
                        ins=[],
                        outs=[],
                        engine=inst.engine,
                        sync_info=mybir.SyncInfo(
                            on_wait=list(inst.sync_info.on_wait), on_update=[]
                        ),
                        text_hint="gather_wait_carrier",
                        bass_nofuse=True,
                    )
                    inst.sync_info.on_wait.clear()
                    new_insts.append(nop)
                    n_moved += 1
                new_insts.append(inst)
            blk.instructions[:] = new_insts


def _wrap_idx(fi_pad):
    """[N] (mult of 16) row ids -> [128, N//16] int16 wrapped+replicated."""
    b16 = fi_pad.reshape(-1, 16)                  # [N/16, 16]
    return np.ascontiguousarray(b16[:, np.arange(P) % 16].T).astype(np.int16)


def _plan(flat_idx, seg):
    """Host-side plan: per-(core, window, pass) token sublists padded to
    global caps so the device program is core-independent."""
    order = np.argsort(seg, kind="stable")
    fi = flat_idx[order].astype(np.int64)
    sg = seg[order].astype(np.int64)
    assert sg.min() >= 0 and sg.max() < B

    lo_mask = fi < SPLIT
    # token sublists for (core k, window w, pass p)
    sub = {}
    counts = np.zeros((NCORES, NW, 2), dtype=np.int64)
    wslot = sg // P                  # global 128-seg window 0..15
    for k in range(NCORES):
        for w in range(NW):
            in_win = wslot == (k * NW + w)
            for p in range(2):
                m = in_win & (lo_mask if p == 0 else ~lo_mask)
                f = fi[m] - (0 if p == 0 else SPLIT)
                s = sg[m] - (k * SEGS_PER_CORE + w * P)   # 0..127
                sub[(k, w, p)] = (f, s)
                counts[k, w, p] = len(f)

    caps = []
    for p in range(2):
        cap = int(counts[:, :, p].max())
        cap = -(-cap // P) * P                    # round up to 128
        caps.append(cap)

    # per-core packed index + segadj arrays (pads gather row 0; excluded
    # from the segment sums by their PAD_SEG segadj entries)
    idx_arrs = [[], []]          # per pass: list per core of wrapped idx
    segadj_arrs = []             # per core: [128, ncols_total] fp16
    ncols = [caps[0] // P, caps[1] // P]
    for k in range(NCORES):
        w_idx = [[], []]
        adj_cols = []
        for w in range(NW):
            for p in range(2):
                f, s = sub[(k, w, p)]
                npad = caps[p] - len(f)
                f = np.concatenate([f, np.zeros(npad, np.int64)])
                s = np.concatenate([s, np.full(npad, PAD_SEG, np.int64)])
                w_idx[p].append(f)
                adj_cols.append(s.reshape(ncols[p], P))
        for p in range(2):
            idx_arrs[p].append(_wrap_idx(np.concatenate(w_idx[p])))
        # segadj layout matches emission order: w0:[A cols..B cols] w1:[...]
        adj = np.concatenate(
            [adj_cols[0], adj_cols[1], adj_cols[2], adj_cols[3]], axis=0
        )                                           # [ncols_total, 128]
        segadj_arrs.append(
            np.ascontiguousarray(adj.T.astype(np.float16))
        )
    return idx_arrs, segadj_arrs, ncols


def _calls(ncol, first_small=False):
    """Split ncol columns into dma_gather calls of <= CALL_COLS columns."""
    out = []
    c = 0
    if first_small and ncol > 1:
        out.append((0, 1))
        c = 1
    while c < ncol:
        n = min(CALL_COLS, ncol - c)
        out.append((c, n))
        c += n
    return out


def _build_program(ncols):
    from concourse import bass, bacc, mybir
    import concourse.tile as tile

    f32 = mybir.dt.float32
    f16 = mybir.dt.float16
    i16 = mybir.dt.int16

    ncols_a, ncols_b = ncols
    ncols_tot = NW * (ncols_a + ncols_b)

    nc = bacc.Bacc()
    tbl_lo = nc.dram_tensor("tbl_lo", [VLO, D], f16, kind="ExternalInput")
    tbl_hi = nc.dram_tensor("tbl_hi", [VHI, D], f16, kind="ExternalInput")
    idx_lo = nc.dram_tensor("idx_lo", [P, NW * ncols_a * 8], i16,
                            kind="ExternalInput")
    idx_hi = nc.dram_tensor("idx_hi", [P, NW * ncols_b * 8], i16,
                            kind="ExternalInput")
    segadj = nc.dram_tensor("segadj", [P, ncols_tot], f16, kind="ExternalInput")
    iota_d = nc.dram_tensor("iota", [P, CALL_COLS * P], f16,
                            kind="ExternalInput")
    ident_d = nc.dram_tensor("ident", [P, P], f32, kind="ExternalInput")
    recip_d = nc.dram_tensor("recip", [NW, P], f32, kind="ExternalInput")
    wpack_d = nc.dram_tensor("wpack", [P, NTCH * DE], f32, kind="ExternalInput")

    out_d = nc.dram_tensor("out", [SEGS_PER_CORE, DE], f32,
                           kind="ExternalOutput")
    import os
    dbg = os.environ.get("KDBG") == "1"
    if dbg:
        dbg_d = nc.dram_tensor("dbg_s", [SEGS_PER_CORE, D], f32,
                               kind="ExternalOutput")

    tbls = [tbl_lo, tbl_hi]
    idx_ds = [idx_lo, idx_hi]

    with tile.TileContext(nc) as tc:
        with (
            tc.tile_pool(name="const", bufs=1) as cpool,
            tc.tile_pool(name="g", bufs=9) as gpool,
            tc.tile_pool(name="e", bufs=6) as epool,
            tc.tile_pool(name="s", bufs=2) as spool,
            tc.tile_pool(name="mt", bufs=2) as mtpool,
            tc.tile_pool(name="osb", bufs=2) as opool,
            tc.tile_pool(name="rc", bufs=2) as rcpool,
            tc.tile_pool(name="pseg", bufs=1, space="PSUM") as pseg_pool,
            tc.tile_pool(name="pc", bufs=1, space="PSUM") as pc_pool,
        ):
            # order matters: the first gather's HWDGE-lane wait transitively
            # covers every earlier const DMA on its lane — load the window-0
            # index tiles first and the big wpack tile last.
            idx_sb = {}
            for w in range(NW):
                for p in range(2):
                    t = cpool.tile([P, ncols[p] * 8], i16, tag=f"idx{w}{p}")
                    o = w * ncols[p] * 8
                    nc.sync.dma_start(out=t[:], in_=idx_ds[p][:, o:o + ncols[p] * 8])
                    idx_sb[(w, p)] = t
                if w == 0:
                    segadj_sb = cpool.tile([P, ncols_tot], f16, tag="segadj")
                    nc.sync.dma_start(out=segadj_sb[:], in_=segadj[:])
                    iota_sb = cpool.tile([P, CALL_COLS, P], f16, tag="iota")
                    nc.sync.dma_start(
                        out=iota_sb[:],
                        in_=iota_d[:].rearrange("p (a b) -> p a b", a=CALL_COLS),
                    )
            ident_sb = cpool.tile([P, P], f32, tag="ident")
            nc.sync.dma_start(out=ident_sb[:], in_=ident_d[:])
            wpack_sb = cpool.tile([P, NTCH * DE], f32, tag="wpack")
            nc.sync.dma_start(out=wpack_sb[:], in_=wpack_d[:])

            ecol = [0]
            for w in range(NW):
                pseg = pseg_pool.tile([P, D], f32, tag="pseg")
                n_win_cols = ncols_a + ncols_b
                col_in_win = 0
                for p in range(2):
                    npc = ncols[p]
                    for (c0, ncall) in _calls(npc, first_small=(w == 0 and p == 0)):
                        g = gpool.tile([P, CALL_COLS, D], f16, tag="g")
                        icol0 = c0 * 8
                        nc.gpsimd.dma_gather(
                            out_ap=g[:, :ncall, :],
                            in_ap=tbls[p][:],
                            idxs_ap=idx_sb[(w, p)][:, icol0:icol0 + ncall * 8],
                            num_idxs=ncall * P,
                            num_idxs_reg=ncall * P,
                            elem_size=D,
                            single_packet=True,
                        )
                        e0 = ecol[0]
                        E = epool.tile([P, CALL_COLS, P], f16, tag="E")
                        nc.vector.tensor_tensor(
                            out=E[:, :ncall, :],
                            in0=segadj_sb[:, e0:e0 + ncall].to_broadcast(
                                [P, ncall, P]),
                            in1=iota_sb[:, :ncall, :],
                            op=mybir.AluOpType.is_equal,
                        )
                        ecol[0] += ncall
                        for t in range(ncall):
                            for ch in range(NCHUNK):
                                nc.tensor.matmul(
                                    out=pseg[:, ch * 512:(ch + 1) * 512],
                                    lhsT=E[:, t, :],
                                    rhs=g[:, t, ch * 512:(ch + 1) * 512],
                                    start=(col_in_win == 0),
                                    stop=(col_in_win == n_win_cols - 1),
                                )
                            col_in_win += 1

                rc = rcpool.tile([P, 1], f32, tag="rc")
                nc.sync.dma_start(out=rc[:], in_=recip_d[w, :, None])
                s = spool.tile([P, D], f32, tag="s")
                nc.vector.tensor_scalar_mul(out=s[:], in0=pseg[:],
                                            scalar1=rc[:, :1])
                if dbg:
                    nc.sync.dma_start(out=dbg_d[w * P:(w + 1) * P, :], in_=s[:])

                po = pseg_pool.tile([P, DE], f32, tag="pseg")
                for cb in range(NTCH // 4):
                    pt = pc_pool.tile([P, 4 * P], f32, tag="pt")
                    for q in range(4):
                        ci = cb * 4 + q
                        nc.tensor.matmul(
                            out=pt[:, q * P:(q + 1) * P],
                            lhsT=s[:, ci * P:(ci + 1) * P],
                            rhs=ident_sb[:],
                            start=(q == 0),
                            stop=True,
                            skip_group_check=True,
                        )
                    mt = mtpool.tile([P, 4 * P], f32, tag="mt")
                    nc.vector.tensor_copy(out=mt[:], in_=pt[:])
                    for q in range(4):
                        ci = cb * 4 + q
                        nc.tensor.matmul(
                            out=po[:],
                            lhsT=mt[:, q * P:(q + 1) * P],
                            rhs=wpack_sb[:, ci * DE:(ci + 1) * DE],
                            start=(ci == 0),
                            stop=(ci == NTCH - 1),
                        )
                osb = opool.tile([P, DE], f32, tag="osb")
                nc.vector.tensor_copy(out=osb[:], in_=po[:])
                nc.sync.dma_start(out=out_d[w * P:(w + 1) * P, :], in_=osb[:])

    nc.compile()
    _move_gather_waits(nc, mybir)
    return nc


def kernel(flat_idx, seg, lens, embed_weight, proj_w, proj_b):
    global LAST_RESULTS
    _ensure_axon_ntff_hook()
    from concourse.bass_utils import run_bass_kernel_spmd

    flat_idx = np.asarray(flat_idx)
    seg = np.asarray(seg)
    lens = np.asarray(lens)
    embed_weight = np.asarray(embed_weight, dtype=np.float32)
    proj_w = np.asarray(proj_w, dtype=np.float32)
    proj_b = np.asarray(proj_b, dtype=np.float32)

    idx_arrs, segadj_arrs, ncols = _plan(flat_idx, seg)
    nc = _build_program(ncols)

    emb16 = embed_weight.astype(np.float16)
    tbl_lo = np.ascontiguousarray(emb16[:SPLIT])
    tbl_hi = np.ascontiguousarray(emb16[SPLIT:])

    iota = np.tile(np.arange(P, dtype=np.float16), (P, CALL_COLS))
    ident = np.eye(P, dtype=np.float32)
    recip_all = (1.0 / lens.astype(np.float64)).astype(np.float32).reshape(
        B // P, P)
    wpack = np.ascontiguousarray(
        proj_w.reshape(NTCH, P, DE).transpose(1, 0, 2).reshape(P, NTCH * DE))
    # wpack[r, ci*DE + e] = proj_w[ci*128 + r, e]

    in_maps = []
    for k in range(NCORES):
        in_maps.append({
            "tbl_lo": tbl_lo,
            "tbl_hi": tbl_hi,
            "idx_lo": idx_arrs[0][k],
            "idx_hi": idx_arrs[1][k],
            "segadj": segadj_arrs[k],
            "iota": iota,
            "ident": ident,
            "recip": recip_all[k * NW:(k + 1) * NW],
            "wpack": wpack,
        })

    res = run_bass_kernel_spmd(nc, in_maps, core_ids=list(range(NCORES)))
    LAST_RESULTS = res

    out = np.empty((B, DE), dtype=np.float32)
    for k in range(NCORES):
        out[k * SEGS_PER_CORE:(k + 1) * SEGS_PER_CORE, :] = (
            res.results[k]["out"])
    out += proj_b
    return out
